# revision 1
# baseline (speedup 1.0000x reference)
import numpy as np

# EnhancedMSTSN — data-parallel over batch across 8 NeuronCores.
# Shapes are fixed by the problem: x [8, 32, 256, 3], params as in setup_inputs().
# Strategy (per sharding hint): shard x on the leading batch axis (1 element per
# core), replicate the tiny parameter set and the N x N adjacency mask (computed
# once on host from params['emb'] — it depends only on parameters, not on x).

B, S, N = 8, 32, 256
EPS_LN = 1e-3


def _adjacency_mask(emb: np.ndarray) -> np.ndarray:
    """Replicates AdaptiveAdjacency from the reference in float32 numpy."""
    emb = emb.astype(np.float32)
    nrm = emb * (1.0 / np.sqrt(np.maximum((emb * emb).sum(-1, keepdims=True), 1e-12)))
    adj = (nrm @ nrm.T).astype(np.float32) * (1.0 - np.eye(N, dtype=np.float32))
    # global top-k(2) threshold
    top2 = np.sort(adj, axis=1)[:, -2:]
    thr = top2.min()
    adj = np.where(adj >= thr, adj, 0.0)
    return (adj > 0.5) | np.eye(N, dtype=bool)


def kernel(x: np.ndarray, params: dict) -> np.ndarray:
    import jax
    import jax.numpy as jnp

    x = np.asarray(x, dtype=np.float32)
    p = jax.tree.map(lambda a: np.asarray(a, dtype=np.float32), params)
    mask_np = _adjacency_mask(np.asarray(p['emb']))

    def _ln(h, g, b):
        m = h.mean(-1, keepdims=True)
        v = ((h - m) ** 2).mean(-1, keepdims=True)
        return (h - m) * jax.lax.rsqrt(v + EPS_LN) * g + b

    def _mha(q_in, kv, Wq, bq, Wk, bk, Wv, bv, Wo, bo):
        scale = np.float32(1.0 / np.sqrt(Wq.shape[-1]))
        q = jnp.einsum('bnd,dhc->bnhc', q_in, Wq) + bq
        k = jnp.einsum('bnd,dhc->bnhc', kv, Wk) + bk
        v = jnp.einsum('bnd,dhc->bnhc', kv, Wv) + bv
        sc = jnp.einsum('bqhc,bkhc->bhqk', q, k) * scale
        a = jax.nn.softmax(sc, axis=-1)
        o = jnp.einsum('bhqk,bkhc->bqhc', a, v)
        return jnp.einsum('bqhc,hco->bqo', o, Wo) + bo

    def _gat(h, mask, gp):
        W, a_s, a_d, bb = gp['W'], gp['a_src'], gp['a_dst'], gp['b']
        Bn, Nn, _ = h.shape
        H, C = a_s.shape
        hf = (h @ W).reshape(Bn, Nn, H, C)
        s = (hf * a_s).sum(-1)
        t = (hf * a_d).sum(-1)
        e = jax.nn.leaky_relu(
            s[:, :, None, :] + t[:, None, :, :], negative_slope=0.2)
        e = jnp.where(mask[None, :, :, None], e, jnp.float32(-1e9))
        alpha = jax.nn.softmax(e, axis=2)
        out = jnp.einsum('bijh,bjhc->bihc', alpha, hf).reshape(Bn, Nn, H * C)
        return out + bb

    def fwd(xb, prm, mask):
        # xb: [1, S, N, 3] — one batch element on this device
        Bq = xb.shape[0]
        xs = xb.reshape(Bq * S, N, 3)
        h = xs @ prm['proj_W'] + prm['proj_b']
        h = jax.nn.relu(_gat(h, mask, prm['gat1']))
        h = _gat(h, mask, prm['gat2'])
        spatial_out = h.reshape(Bq, S, N, 32)
        t_in = spatial_out.reshape(Bq * N, S, 32)
        attn = _mha(t_in, t_in, prm['tWq'], prm['tbq'], prm['tWk'], prm['tbk'],
                    prm['tWv'], prm['tbv'], prm['tWo'], prm['tbo'])
        out1 = _ln(t_in + attn, prm['ln1_g'], prm['ln1_b'])
        ffn = jax.nn.gelu(out1 @ prm['fW1'] + prm['fb1'],
                          approximate=False) @ prm['fW2'] + prm['fb2']
        t_out = _ln(out1 + ffn, prm['ln2_g'], prm['ln2_b'])
        temporal_out = t_out.reshape(Bq, N, S, 32)
        spatial_feats = spatial_out.mean(axis=1)
        temporal_feats = temporal_out.mean(axis=2)
        fused = _mha(spatial_feats, temporal_feats,
                     prm['cWq'], prm['cbq'], prm['cWk'], prm['cbk'],
                     prm['cWv'], prm['cbv'], prm['cWo'], prm['cbo'])
        hid = jax.nn.gelu(fused @ prm['rW1'] + prm['rb1'], approximate=False)
        return (hid @ prm['rW2'] + prm['rb2'])[..., 0]

    n_dev = min(8, jax.device_count())
    assert B % n_dev == 0
    xsh = x.reshape(n_dev, B // n_dev, S, N, 3)
    mask = jnp.asarray(mask_np)
    run = jax.pmap(fwd, in_axes=(0, None, None))
    out = run(xsh, p, mask)  # [n_dev, B/n_dev, N]
    return np.asarray(out).reshape(B, N).astype(np.float32)


if __name__ == '__main__':
    import reference
    ins = reference.setup_inputs()
    got = kernel(**{k: np.asarray(v) if not isinstance(v, dict) else v
                    for k, v in ins.items()})
    print(got.shape, got.dtype)
